# revision 10
# baseline (speedup 1.0000x reference)
import sys

for _p in ("/opt/trn_rl_repo",):
    if _p not in sys.path:
        sys.path.insert(0, _p)

import numpy as np
import ml_dtypes
import bass_rust
import concourse.bass as bass
import concourse.mybir as mybir
import concourse.tile as tile
from concourse.bass_utils import run_bass_kernel_spmd

DT = mybir.dt
F8 = ml_dtypes.float8_e4m3
DR = mybir.MatmulPerfMode.DoubleRow

# Problem constants (hardcoded from the nn_AutoFlow spec)
B, D, NH0, NH1, L = 32768, 64, 256, 256, 16
NCORES = 8
BC = B // NCORES          # 4096 samples per core
BT = 512                  # batch tile (free dim of activation tiles)
NT = BC // BT             # tiles per core
NP = NT // 2              # tile pairs: tile 2p -> partitions 0:64, 2p+1 -> 64:128
WARM_MMS = 40
WARM_N = 128

# fp8 weight blob column layout, per layer (DoubleRow pairs flattened as
# (k m) so rearrange("p (k m) -> p k m", k=2) recovers the pair)
# mm0 net n chunk m: [64p, 2x128] = (A0 m-chunk | b0 row) at n*512 + m*256
#   (rows 64:128 hold a copy for odd tiles whose y sits at partitions 64:128)
# mm1 net n chunk m: [128p, 2x128] = (A1 k0 mcols | A1 k1) at 1024+n*512+m*256
# mm2 net n:         [128p, 2x64]  = (A2 k0 | A2 k1)       at 2048 + n*128
CW8 = 2304

# fp16 blob: per layer col 0:64 = -b2l row (partitions 0 and 64)
CW16 = 64

# const fp16 blob [128, 576]: cols 0:64 = I64 (rows 0:64 and 64:128),
# cols 64:576 = 1.0
CONST_COLS = 576

# fp32 bias blob, per layer 5 cols:
# 0: b1_loc[0:128]  1: b1_loc[128:256]  2: b1_sc[0:128]  3: b1_sc[128:256]
# 4: -b2s (rows 0:64 and 64:128)
CB = 5

# engines for the per-(layer,tile) psum-draining ops. Only ACT ("act") and
# DVE ("dve") may touch PSUM; gpsimd/Pool is SBUF-only on this stack.
RELU0_ENG = {(0, 0): "act", (0, 1): "act", (1, 0): "dve", (1, 1): "dve"}
RELU1_ENG = {(0, 0): "act", (0, 1): "dve", (1, 0): "dve", (1, 1): "dve"}
NY16_ENG = "dve"


def _patch_tile_drain(maxw=1):
    """walrus on this stack allows only 1 sync-wait on the kernel-tail Drain;
    split the TileContext drain's waits across a chain of drains."""
    from concourse.tile import ScopedClock

    def _drain_and_barrier(self, tick_clock, wait_clock):
        drain_inst = self.nc.sync.drain()
        wait_clock.add_sem_waits(
            drain_inst.ins, ScopedClock({None: tick_clock.global_clock})
        )
        inst = drain_inst.ins
        si = inst.sync_info
        if si is not None:
            waits = list(si.on_wait)
            ups = list(si.on_update)
            if len(waits) > maxw:
                chunks = [waits[i:i + maxw] for i in range(0, len(waits), maxw)]
                inst.sync_info = bass_rust.SyncInfo(on_wait=chunks[0], on_update=[])
                for j, chunk in enumerate(chunks[1:]):
                    extra = self.nc.sync.drain().ins
                    is_last = j == len(chunks) - 2
                    extra.sync_info = bass_rust.SyncInfo(
                        on_wait=chunk, on_update=ups if is_last else []
                    )
        self.nc.all_engine_barrier()
        assert self.sems is not None
        popped = self.nc._tile_sem_poison_stack.pop()
        assert popped is self._sem_poison
        self.nc.clear_and_free_semaphores(list(self.sems.allocated().values()))
        self.nc.all_engine_barrier()

    tile.TileContext._drain_and_barrier = _drain_and_barrier


_MAXW1_TYPES = ("InstDrain", "InstActivation")


def _split_excess_waits(nc, maxw=1):
    """walrus on this stack encodes very few semaphore-wait slots per
    instruction. Spill excess waits onto same-engine NoOps inserted just
    before the instruction (engine streams are in-order, so this is
    equivalent)."""
    for f in nc.m.functions:
        for bb in f.blocks:
            il = bb.instructions
            out = []
            for inst in il:
                si = getattr(inst, "sync_info", None)
                mw = 1 if type(inst).__name__ in _MAXW1_TYPES else maxw
                if si is not None and len(si.on_wait) > mw:
                    waits = list(si.on_wait)
                    ups = list(si.on_update)
                    chunks = [waits[i:i + mw] for i in range(0, len(waits), mw)]
                    for j, ch in enumerate(chunks[:-1]):
                        nop = mybir.InstNoOp(
                            name=f"{inst.name}-wsp{j}", ins=[], outs=[]
                        )
                        nop.engine = inst.engine
                        nop.sync_info = bass_rust.SyncInfo(on_wait=ch, on_update=[])
                        nc.register_instruction(nop, overwrite=True)
                        out.append(nop)
                    inst.sync_info = bass_rust.SyncInfo(
                        on_wait=chunks[-1], on_update=ups
                    )
                out.append(inst)
            if len(out) != len(il):
                il[:] = out


def _build_masks():
    mh0 = np.arange(NH0) % (D - 1)
    mh1 = np.arange(NH1) % (D - 1)
    M1 = (mh0[None, :] <= mh1[:, None]).astype(np.float32)
    M0s, M2s = [], []
    for l in range(L):
        perm = np.arange(D) if l % 2 == 0 else np.arange(D)[::-1]
        M0s.append((perm[None, :] <= mh0[:, None]).astype(np.float32))
        M2s.append((mh1[None, :] < perm[:, None]).astype(np.float32))
    return np.stack(M0s), np.broadcast_to(M1, (L,) + M1.shape).copy(), np.stack(M2s)


PRIO_GROUP = 8


def _PRIO(l, ph, t):
    g, tin = t // PRIO_GROUP, t % PRIO_GROUP
    return (((l * (NT // PRIO_GROUP) + g) * 8 + ph) * PRIO_GROUP + tin) * 64


def _relu(nc, eng, out_ap, in_ap, bias_ap=None):
    AF = mybir.ActivationFunctionType
    ALU = mybir.AluOpType
    if eng == "act":
        nc.scalar.activation(out_ap, in_ap, AF.Relu,
                             bias=0.0 if bias_ap is None else bias_ap)
    else:
        if bias_ap is None:
            nc.vector.tensor_scalar(out_ap, in_ap, 0.0, None, ALU.max)
        else:
            nc.vector.tensor_scalar(out_ap, in_ap, bias_ap, 0.0,
                                    ALU.add, ALU.max)


def _emit_layer(nc, tc, pools, tiles, l, t, last):
    f16, f32, f8 = DT.float16, DT.float32, DT.float8e4
    AF = mybir.ActivationFunctionType
    ALU = mybir.AluOpType
    hpool, lppool, h8pool, epool, opool, pairst = pools
    wt8, wt16, ct, bt, y16s, y8s = tiles
    c8 = l * CW8
    cb = l * CB
    p, q = t >> 1, t & 1          # pair index, parity (partition half)
    pb = 64 * q                   # partition base of this tile's [64,*] data

    def pair8(off, w):
        return wt8[0:128, c8 + off: c8 + off + 2 * w].rearrange(
            "p (k m) -> p k m", k=2)

    # ph0: mm0 both nets via DoubleRow (y8 | ones) x (A0 | b0row)
    tc.cur_priority = _PRIO(l, 0, t)
    ps0 = {}
    for n in (0, 1):
        for m in (0, 1):
            pp = hpool.tile([128, BT], f32, tag="hp", name=f"ps0_{n}{m}")
            off = c8 + n * 512 + m * 256
            lhsT = wt8[pb:pb + 64, off: off + 256].rearrange(
                "p (k m) -> p k m", k=2)
            nc.tensor.matmul(pp[:, :], lhsT, y8s[p][pb:pb + 64, :, :],
                             start=True, stop=True, perf_mode=DR)
            ps0[(n, m)] = pp

    # ph1: relu0 (b0 already in psum via the ones slot)
    tc.cur_priority = _PRIO(l, 1, t)
    h8 = {}
    for n in (0, 1):
        h8[n] = h8pool.tile([128, 2, BT], f8, tag="h8", name=f"h8_{n}")
        for m in (0, 1):
            _relu(nc, RELU0_ENG[(n, m)], h8[n][:, m, :], ps0[(n, m)][:, :])

    # ph2: mm1 DoubleRow K=256
    tc.cur_priority = _PRIO(l, 2, t)
    ps1 = {}
    for n in (0, 1):
        for m in (0, 1):
            pp = hpool.tile([128, BT], f32, tag="hp", name=f"ps1_{n}{m}")
            lhsT = pair8(1024 + n * 512 + m * 256, 128)
            nc.tensor.matmul(pp[:, :], lhsT, h8[n][:, :, :],
                             start=True, stop=True, perf_mode=DR)
            ps1[(n, m)] = pp

    # ph3: relu1 with b1 bias
    tc.cur_priority = _PRIO(l, 3, t)
    h18 = {}
    for n in (0, 1):
        h18[n] = h8pool.tile([128, 2, BT], f8, tag="h18", name=f"h18_{n}")
        for m in (0, 1):
            bias_ap = bt[:, cb + n * 2 + m: cb + n * 2 + m + 1]
            _relu(nc, RELU1_ENG[(n, m)], h18[n][:, m, :], ps1[(n, m)][:, :],
                  bias_ap)

    # ph4: mm2 into the pair's shared psum banks. Even tile -> rows 0:64
    # (DoubleRow), odd tile -> rows 64:128 (plain matmuls: DR can't write
    # partition base 64). loc psum accumulates -b2l (K=1) and +y16 (identity)
    # so it holds t = y - loc - b2l when done.
    tc.cur_priority = _PRIO(l, 4, t)
    if q == 0:
        psc = lppool.tile([128, BT], f32, tag="lsc")
        pt = lppool.tile([128, BT], f32, tag="lt")
        pairst[p] = (psc, pt)
        nc.tensor.matmul(psc[0:64, :], pair8(2048 + 128, 64), h18[1][:, :, :],
                         start=True, stop=True, perf_mode=DR)
        nc.tensor.matmul(pt[0:64, :], wt16[0:1, l * CW16: l * CW16 + 64],
                         ct[0:1, 64: 64 + BT], start=True, stop=False)
        nc.tensor.matmul(pt[0:64, :], pair8(2048, 64), h18[0][:, :, :],
                         start=False, stop=False, perf_mode=DR)
        nc.tensor.matmul(pt[0:64, :], ct[0:64, 0:64], y16s[p][0:64, :],
                         start=False, stop=True)
        return None
    psc, pt = pairst[p]
    # sc net (plain fp8 matmuls, k-chunks accumulated)
    soff = c8 + 2048 + 128
    for k in (0, 1):
        nc.tensor.matmul(psc[64:128, :],
                         wt8[0:128, soff + 64 * k: soff + 64 * (k + 1)],
                         h18[1][:, k, :], start=(k == 0), stop=(k == 1))
    # loc net: -b2l, -loc (2 plain chunks), +y16
    loff = c8 + 2048
    nc.tensor.matmul(pt[64:128, :], wt16[64:65, l * CW16: l * CW16 + 64],
                     ct[64:65, 64: 64 + BT], start=True, stop=False)
    for k in (0, 1):
        nc.tensor.matmul(pt[64:128, :],
                         wt8[0:128, loff + 64 * k: loff + 64 * (k + 1)],
                         h18[0][:, k, :], start=False, stop=False)
    nc.tensor.matmul(pt[64:128, :], ct[64:128, 0:64], y16s[p][64:128, :],
                     start=False, stop=True)

    # ph5: paired coupling over both tiles: e = exp(-sc - b2s); y' = t * e
    tc.cur_priority = _PRIO(l, 5, t)
    e16 = epool.tile([128, BT], f16, tag="e")
    nc.scalar.activation(e16[:], psc[:, :], AF.Exp,
                         bias=bt[:, cb + 4: cb + 5], scale=-1.0)
    if last:
        o32 = opool.tile([128, BT], f32, tag="o32")
        nc.vector.tensor_tensor(o32[:], pt[:, :], e16[:], ALU.mult)
        return o32
    nc.vector.tensor_tensor(y16s[p][:, :], pt[:, :], e16[:], ALU.mult)
    # fp8 copy for the next layer's mm0 via casting DMA (off the engines)
    nc.gpsimd.dma_start(y8s[p][:, 0, :], y16s[p][:, :])
    return None


def _build():
    _patch_tile_drain(1)
    from contextlib import ExitStack

    f16, f32, f8 = DT.float16, DT.float32, DT.float8e4
    nc = bass.Bass(target_bir_lowering=False)
    u16_d = nc.declare_dram_parameter("u16", [64, BC], f16, isOutput=False)
    u8_d = nc.declare_dram_parameter("u8", [64, BC], f8, isOutput=False)
    w8_d = nc.declare_dram_parameter("w8", [L, 128, CW8], f8, isOutput=False)
    w16_d = nc.declare_dram_parameter("w16", [128, L * CW16], f16, isOutput=False)
    cn_d = nc.declare_dram_parameter("cn", [128, CONST_COLS], f16, isOutput=False)
    b_d = nc.declare_dram_parameter("bias", [128, L * CB], f32, isOutput=False)
    out_d = nc.declare_dram_parameter("out", [64, BC], f32, isOutput=True)

    with tile.TileContext(nc) as tc, ExitStack() as ctx:
        wpool = ctx.enter_context(tc.tile_pool(name="w", bufs=1))
        hpool = ctx.enter_context(tc.tile_pool(name="hp", bufs=4, space="PSUM"))
        lppool = ctx.enter_context(tc.tile_pool(name="lp", bufs=2, space="PSUM"))
        h8pool = ctx.enter_context(tc.tile_pool(name="h8", bufs=6))
        epool = ctx.enter_context(tc.tile_pool(name="e", bufs=4))
        opool = ctx.enter_context(tc.tile_pool(name="o", bufs=3))

        wt8 = wpool.tile([128, L * CW8], f8)
        wt16 = wpool.tile([128, L * CW16], f16)
        ct = wpool.tile([128, CONST_COLS], f16)
        bt = wpool.tile([128, L * CB], f32)

        # PE warmup: keep the clock-ramp monitor busy while DMAs land
        warm = wpool.tile([128, 128], f16)
        wps = hpool.tile([128, WARM_N], f32, tag="hp")
        nc.gpsimd.memset(warm[:], 0.0)
        for _ in range(WARM_MMS):
            nc.tensor.matmul(wps[:, 0:WARM_N], warm[:, 0:WARM_N],
                             warm[:, 0:WARM_N], start=True, stop=True)

        nc.sync.dma_start(wt8[:, 0:CW8], w8_d[0])
        nc.sync.dma_start(ct[:], cn_d[:])
        nc.sync.dma_start(wt16[:], w16_d[:])
        nc.sync.dma_start(bt[:], b_d[:])

        y16s, y8s = [], []
        for p in range(NP):
            yt = wpool.tile([128, BT], f16, name=f"y16_{p}")
            nc.sync.dma_start(yt[0:64, :], u16_d[:, (2 * p) * BT:(2 * p + 1) * BT])
            nc.sync.dma_start(yt[64:128, :],
                              u16_d[:, (2 * p + 1) * BT:(2 * p + 2) * BT])
            y16s.append(yt)
        for p in range(NP):
            yt = wpool.tile([128, 2, BT], f8, name=f"y8_{p}")
            nc.sync.dma_start(yt[0:64, 0, :], u8_d[:, (2 * p) * BT:(2 * p + 1) * BT])
            nc.sync.dma_start(yt[64:128, 0, :],
                              u8_d[:, (2 * p + 1) * BT:(2 * p + 2) * BT])
            nc.gpsimd.memset(yt[:, 1, :], 1.0)
            y8s.append(yt)
        for l in range(1, L):
            nc.sync.dma_start(wt8[:, l * CW8:(l + 1) * CW8], w8_d[l])

        pairst = {}
        pools = (hpool, lppool, h8pool, epool, opool, pairst)
        tiles = (wt8, wt16, ct, bt, y16s, y8s)
        for l in range(L):
            for t in range(NT):
                o32 = _emit_layer(nc, tc, pools, tiles, l, t, l == L - 1)
                if o32 is not None:
                    p = t >> 1
                    nc.sync.dma_start(out_d[:, (2 * p) * BT:(2 * p + 1) * BT],
                                      o32[0:64, :])
                    nc.sync.dma_start(out_d[:, (2 * p + 1) * BT:(2 * p + 2) * BT],
                                      o32[64:128, :])
    _split_excess_waits(nc, maxw=1)
    return nc


_NC_CACHE = None


def _prep_blobs(inputs):
    M0, M1, M2 = _build_masks()
    w8 = np.zeros((L, 128, CW8), F8)
    w16 = np.zeros((128, L * CW16), np.float16)
    cn = np.zeros((128, CONST_COLS), np.float16)
    bb = np.zeros((128, L * CB), np.float32)
    cn[0:64, 0:64] = np.eye(64, dtype=np.float16)
    cn[64:128, 0:64] = np.eye(64, dtype=np.float16)
    cn[:, 64:CONST_COLS] = 1.0
    for l in range(L):
        for n, name in ((0, "loc"), (1, "scale")):
            A0 = (M0[l] * inputs[f"{name}_W0"][l]).astype(np.float32).T  # [64,256]
            A1 = (M1[l] * inputs[f"{name}_W1"][l]).astype(np.float32).T  # [256,256]
            A2 = (M2[l] * inputs[f"{name}_W2"][l]).astype(np.float32).T  # [256,64]
            b0 = inputs[f"{name}_b0"][l].astype(np.float32)
            b1 = inputs[f"{name}_b1"][l].astype(np.float32)
            b2 = inputs[f"{name}_b2"][l].astype(np.float32)
            if n == 0:
                A2 = -A2
                w16[0, l * CW16: l * CW16 + 64] = -b2.astype(np.float16)
                w16[64, l * CW16: l * CW16 + 64] = -b2.astype(np.float16)
            else:
                bb[0:64, l * CB + 4] = -b2
                bb[64:128, l * CB + 4] = -b2
            for m in (0, 1):
                off = n * 512 + m * 256
                a0c = A0[:, m * 128:(m + 1) * 128].astype(F8)
                w8[l, 0:64, off: off + 128] = a0c
                w8[l, 64:128, off: off + 128] = a0c
                b0c = b0[m * 128:(m + 1) * 128].astype(F8)
                w8[l, 0, off + 128: off + 256] = b0c
                w8[l, 64, off + 128: off + 256] = b0c
                off = 1024 + n * 512 + m * 256
                w8[l, :, off: off + 128] = \
                    A1[0:128, m * 128:(m + 1) * 128].astype(F8)
                w8[l, :, off + 128: off + 256] = \
                    A1[128:256, m * 128:(m + 1) * 128].astype(F8)
                bb[:, l * CB + n * 2 + m] = b1[m * 128:(m + 1) * 128]
            off = 2048 + n * 128
            w8[l, :, off: off + 64] = A2[0:128, :].astype(F8)
            w8[l, :, off + 64: off + 128] = A2[128:256, :].astype(F8)
    return w8, w16, cn, bb


def make_in_maps(inputs):
    inputs = {k: np.asarray(v) for k, v in inputs.items()}
    u = inputs["u"].astype(np.float32)            # [B, 64]
    w8, w16, cn, bb = _prep_blobs(inputs)
    uT16 = np.ascontiguousarray(u.T).astype(np.float16)
    uT8 = uT16.astype(F8)
    in_maps = []
    for c in range(NCORES):
        sl = slice(c * BC, (c + 1) * BC)
        in_maps.append({
            "u16": np.ascontiguousarray(uT16[:, sl]),
            "u8": np.ascontiguousarray(uT8[:, sl]),
            "w8": w8, "w16": w16, "cn": cn, "bias": bb,
        })
    return in_maps


def kernel(**inputs):
    global _NC_CACHE
    if _NC_CACHE is None:
        _NC_CACHE = _build()
    nc = _NC_CACHE
    in_maps = make_in_maps(inputs)
    res = run_bass_kernel_spmd(nc, in_maps, core_ids=list(range(NCORES)))
    out = np.empty((64, B), np.float32)
    for c in range(NCORES):
        out[:, c * BC:(c + 1) * BC] = res.results[c]["out"]
    return np.ascontiguousarray(out.T)


# revision 11
# speedup vs baseline: 1.1182x; 1.1182x over previous
import sys

for _p in ("/opt/trn_rl_repo",):
    if _p not in sys.path:
        sys.path.insert(0, _p)

import numpy as np
import ml_dtypes
import bass_rust
import concourse.bass as bass
import concourse.mybir as mybir
import concourse.tile as tile
from concourse.bass_utils import run_bass_kernel_spmd

DT = mybir.dt
F8 = ml_dtypes.float8_e4m3
DR = mybir.MatmulPerfMode.DoubleRow

# Problem constants (hardcoded from the nn_AutoFlow spec)
B, D, NH0, NH1, L = 32768, 64, 256, 256, 16
NCORES = 8
BC = B // NCORES          # 4096 samples per core
BT = 512                  # batch tile (free dim of activation tiles)
NT = BC // BT             # tiles per core
NP = NT // 2              # tile pairs: tile 2p -> partitions 0:64, 2p+1 -> 64:128
WARM_MMS = 40
WARM_N = 128

# fp8 weight blob column layout, per layer (DoubleRow pairs flattened as
# (k m) so rearrange("p (k m) -> p k m", k=2) recovers the pair)
# mm0 net n chunk m: [64p, 2x128] = (A0 m-chunk | b0 row) at n*512 + m*256
#   (rows 64:128 hold a copy for odd tiles whose y sits at partitions 64:128)
# mm1 net n chunk m: [128p, 2x128] = (A1 k0 mcols | A1 k1) at 1024+n*512+m*256
# mm2 net n:         [128p, 2x64]  = (A2 k0 | A2 k1)       at 2048 + n*128
CW8 = 2304

# fp16 blob: per layer col 0:64 = -b2l row (partitions 0 and 64)
CW16 = 64

# const fp16 blob [128, 576]: cols 0:64 = I64 (rows 0:64 and 64:128),
# cols 64:576 = 1.0
CONST_COLS = 576

# fp32 bias blob, per layer 5 cols:
# 0: b1_loc[0:128]  1: b1_loc[128:256]  2: b1_sc[0:128]  3: b1_sc[128:256]
# 4: -b2s (rows 0:64 and 64:128)
CB = 5

# engines for the per-(layer,tile) psum-draining ops. Only ACT ("act") and
# DVE ("dve") may touch PSUM; gpsimd/Pool is SBUF-only on this stack.
RELU0_ENG = {(0, 0): "act", (0, 1): "act", (1, 0): "dve", (1, 1): "split"}
RELU1_ENG = {(0, 0): "act", (0, 1): "act", (1, 0): "dve", (1, 1): "dve"}
RELU_SPLIT_C = 135   # "split": cols [0:C] on ACT, [C:BT] on DVE


def _patch_tile_drain(maxw=1):
    """walrus on this stack allows only 1 sync-wait on the kernel-tail Drain;
    split the TileContext drain's waits across a chain of drains."""
    from concourse.tile import ScopedClock

    def _drain_and_barrier(self, tick_clock, wait_clock):
        drain_inst = self.nc.sync.drain()
        wait_clock.add_sem_waits(
            drain_inst.ins, ScopedClock({None: tick_clock.global_clock})
        )
        inst = drain_inst.ins
        si = inst.sync_info
        if si is not None:
            waits = list(si.on_wait)
            ups = list(si.on_update)
            if len(waits) > maxw:
                chunks = [waits[i:i + maxw] for i in range(0, len(waits), maxw)]
                inst.sync_info = bass_rust.SyncInfo(on_wait=chunks[0], on_update=[])
                for j, chunk in enumerate(chunks[1:]):
                    extra = self.nc.sync.drain().ins
                    is_last = j == len(chunks) - 2
                    extra.sync_info = bass_rust.SyncInfo(
                        on_wait=chunk, on_update=ups if is_last else []
                    )
        self.nc.all_engine_barrier()
        assert self.sems is not None
        popped = self.nc._tile_sem_poison_stack.pop()
        assert popped is self._sem_poison
        self.nc.clear_and_free_semaphores(list(self.sems.allocated().values()))
        self.nc.all_engine_barrier()

    tile.TileContext._drain_and_barrier = _drain_and_barrier


_MAXW1_TYPES = ("InstDrain", "InstActivation")


def _split_excess_waits(nc, maxw=1):
    """walrus on this stack encodes very few semaphore-wait slots per
    instruction. Spill excess waits onto same-engine NoOps inserted just
    before the instruction (engine streams are in-order, so this is
    equivalent)."""
    for f in nc.m.functions:
        for bb in f.blocks:
            il = bb.instructions
            out = []
            for inst in il:
                si = getattr(inst, "sync_info", None)
                mw = 1 if type(inst).__name__ in _MAXW1_TYPES else maxw
                if si is not None and len(si.on_wait) > mw:
                    waits = list(si.on_wait)
                    ups = list(si.on_update)
                    chunks = [waits[i:i + mw] for i in range(0, len(waits), mw)]
                    for j, ch in enumerate(chunks[:-1]):
                        nop = mybir.InstNoOp(
                            name=f"{inst.name}-wsp{j}", ins=[], outs=[]
                        )
                        nop.engine = inst.engine
                        nop.sync_info = bass_rust.SyncInfo(on_wait=ch, on_update=[])
                        nc.register_instruction(nop, overwrite=True)
                        out.append(nop)
                    inst.sync_info = bass_rust.SyncInfo(
                        on_wait=chunks[-1], on_update=ups
                    )
                out.append(inst)
            if len(out) != len(il):
                il[:] = out


def _build_masks():
    mh0 = np.arange(NH0) % (D - 1)
    mh1 = np.arange(NH1) % (D - 1)
    M1 = (mh0[None, :] <= mh1[:, None]).astype(np.float32)
    M0s, M2s = [], []
    for l in range(L):
        perm = np.arange(D) if l % 2 == 0 else np.arange(D)[::-1]
        M0s.append((perm[None, :] <= mh0[:, None]).astype(np.float32))
        M2s.append((mh1[None, :] < perm[:, None]).astype(np.float32))
    return np.stack(M0s), np.broadcast_to(M1, (L,) + M1.shape).copy(), np.stack(M2s)


PRIO_GROUP = 8


def _PRIO(l, ph, t):
    g, tin = t // PRIO_GROUP, t % PRIO_GROUP
    return (((l * (NT // PRIO_GROUP) + g) * 8 + ph) * PRIO_GROUP + tin) * 64


def _relu(nc, eng, out_ap, in_ap, bias_ap=None):
    AF = mybir.ActivationFunctionType
    ALU = mybir.AluOpType
    if eng == "split":
        c = RELU_SPLIT_C
        _relu(nc, "act", out_ap[:, 0:c], in_ap[:, 0:c], bias_ap)
        _relu(nc, "dve", out_ap[:, c:BT], in_ap[:, c:BT], bias_ap)
        return
    if eng == "act":
        nc.scalar.activation(out_ap, in_ap, AF.Relu,
                             bias=0.0 if bias_ap is None else bias_ap)
    else:
        if bias_ap is None:
            nc.vector.tensor_scalar(out_ap, in_ap, 0.0, None, ALU.max)
        else:
            nc.vector.tensor_scalar(out_ap, in_ap, bias_ap, 0.0,
                                    ALU.add, ALU.max)


def _emit_layer(nc, tc, pools, tiles, l, t, last):
    f16, f32, f8 = DT.float16, DT.float32, DT.float8e4
    AF = mybir.ActivationFunctionType
    ALU = mybir.AluOpType
    hpool, lppool, h8pool, epool, opool, pairst = pools
    wt8, wt16, ct, bt, y16s, y8s = tiles
    c8 = l * CW8
    cb = l * CB
    p, q = t >> 1, t & 1          # pair index, parity (partition half)
    pb = 64 * q                   # partition base of this tile's [64,*] data

    def pair8(off, w):
        return wt8[0:128, c8 + off: c8 + off + 2 * w].rearrange(
            "p (k m) -> p k m", k=2)

    # ph0: mm0 both nets via DoubleRow (y8 | ones) x (A0 | b0row)
    tc.cur_priority = _PRIO(l, 0, t)
    ps0 = {}
    for n in (0, 1):
        for m in (0, 1):
            pp = hpool.tile([128, BT], f32, tag="hp", name=f"ps0_{n}{m}")
            off = c8 + n * 512 + m * 256
            lhsT = wt8[pb:pb + 64, off: off + 256].rearrange(
                "p (k m) -> p k m", k=2)
            nc.tensor.matmul(pp[:, :], lhsT, y8s[p][pb:pb + 64, :, :],
                             start=True, stop=True, perf_mode=DR)
            ps0[(n, m)] = pp

    # ph1: relu0 (b0 already in psum via the ones slot)
    tc.cur_priority = _PRIO(l, 1, t)
    h8 = {}
    for n in (0, 1):
        h8[n] = h8pool.tile([128, 2, BT], f8, tag="h8", name=f"h8_{n}")
        for m in (0, 1):
            _relu(nc, RELU0_ENG[(n, m)], h8[n][:, m, :], ps0[(n, m)][:, :])

    # ph2: mm1 DoubleRow K=256
    tc.cur_priority = _PRIO(l, 2, t)
    ps1 = {}
    for n in (0, 1):
        for m in (0, 1):
            pp = hpool.tile([128, BT], f32, tag="hp", name=f"ps1_{n}{m}")
            lhsT = pair8(1024 + n * 512 + m * 256, 128)
            nc.tensor.matmul(pp[:, :], lhsT, h8[n][:, :, :],
                             start=True, stop=True, perf_mode=DR)
            ps1[(n, m)] = pp

    # ph3: relu1 with b1 bias
    tc.cur_priority = _PRIO(l, 3, t)
    h18 = {}
    for n in (0, 1):
        h18[n] = h8pool.tile([128, 2, BT], f8, tag="h18", name=f"h18_{n}")
        for m in (0, 1):
            bias_ap = bt[:, cb + n * 2 + m: cb + n * 2 + m + 1]
            _relu(nc, RELU1_ENG[(n, m)], h18[n][:, m, :], ps1[(n, m)][:, :],
                  bias_ap)

    # ph4: mm2 into the pair's shared psum banks. Even tile -> rows 0:64
    # (DoubleRow), odd tile -> rows 64:128 (plain matmuls: DR can't write
    # partition base 64). loc psum accumulates -b2l (K=1) and +y16 (identity)
    # so it holds t = y - loc - b2l when done.
    tc.cur_priority = _PRIO(l, 4, t)
    if q == 0:
        psc = lppool.tile([128, BT], f32, tag="lsc")
        pt = lppool.tile([128, BT], f32, tag="lt")
        pairst[p] = (psc, pt)
        nc.tensor.matmul(psc[0:64, :], pair8(2048 + 128, 64), h18[1][:, :, :],
                         start=True, stop=True, perf_mode=DR)
        nc.tensor.matmul(pt[0:64, :], wt16[0:1, l * CW16: l * CW16 + 64],
                         ct[0:1, 64: 64 + BT], start=True, stop=False)
        nc.tensor.matmul(pt[0:64, :], pair8(2048, 64), h18[0][:, :, :],
                         start=False, stop=False, perf_mode=DR)
        nc.tensor.matmul(pt[0:64, :], ct[0:64, 0:64], y16s[p][0:64, :],
                         start=False, stop=True)
        return None
    psc, pt = pairst[p]
    # sc net (plain fp8 matmuls, k-chunks accumulated)
    soff = c8 + 2048 + 128
    for k in (0, 1):
        nc.tensor.matmul(psc[64:128, :],
                         wt8[0:128, soff + 64 * k: soff + 64 * (k + 1)],
                         h18[1][:, k, :], start=(k == 0), stop=(k == 1))
    # loc net: -b2l, -loc (2 plain chunks), +y16
    loff = c8 + 2048
    nc.tensor.matmul(pt[64:128, :], wt16[64:65, l * CW16: l * CW16 + 64],
                     ct[64:65, 64: 64 + BT], start=True, stop=False)
    for k in (0, 1):
        nc.tensor.matmul(pt[64:128, :],
                         wt8[0:128, loff + 64 * k: loff + 64 * (k + 1)],
                         h18[0][:, k, :], start=False, stop=False)
    nc.tensor.matmul(pt[64:128, :], ct[64:128, 0:64], y16s[p][64:128, :],
                     start=False, stop=True)

    # ph5: paired coupling over both tiles: e = exp(-sc - b2s); y' = t * e
    tc.cur_priority = _PRIO(l, 5, t)
    e16 = epool.tile([128, BT], f16, tag="e")
    nc.scalar.activation(e16[:], psc[:, :], AF.Exp,
                         bias=bt[:, cb + 4: cb + 5], scale=-1.0)
    if last:
        o32 = opool.tile([128, BT], f32, tag="o32")
        nc.vector.tensor_tensor(o32[:], pt[:, :], e16[:], ALU.mult)
        return o32
    nc.vector.tensor_tensor(y16s[p][:, :], pt[:, :], e16[:], ALU.mult)
    # fp8 copy for the next layer's mm0 via casting DMA (off the engines)
    nc.gpsimd.dma_start(y8s[p][:, 0, :], y16s[p][:, :])
    return None


def _build():
    _patch_tile_drain(1)
    from contextlib import ExitStack

    f16, f32, f8 = DT.float16, DT.float32, DT.float8e4
    nc = bass.Bass(target_bir_lowering=False)
    u16_d = nc.declare_dram_parameter("u16", [64, BC], f16, isOutput=False)
    u8_d = nc.declare_dram_parameter("u8", [64, BC], f8, isOutput=False)
    w8_d = nc.declare_dram_parameter("w8", [L, 128, CW8], f8, isOutput=False)
    w16_d = nc.declare_dram_parameter("w16", [128, L * CW16], f16, isOutput=False)
    cn_d = nc.declare_dram_parameter("cn", [128, CONST_COLS], f16, isOutput=False)
    b_d = nc.declare_dram_parameter("bias", [128, L * CB], f32, isOutput=False)
    out_d = nc.declare_dram_parameter("out", [64, BC], f32, isOutput=True)

    with tile.TileContext(nc) as tc, ExitStack() as ctx:
        wpool = ctx.enter_context(tc.tile_pool(name="w", bufs=1))
        hpool = ctx.enter_context(tc.tile_pool(name="hp", bufs=4, space="PSUM"))
        lppool = ctx.enter_context(tc.tile_pool(name="lp", bufs=2, space="PSUM"))
        h8pool = ctx.enter_context(tc.tile_pool(name="h8", bufs=6))
        epool = ctx.enter_context(tc.tile_pool(name="e", bufs=4))
        opool = ctx.enter_context(tc.tile_pool(name="o", bufs=3))

        wt8 = wpool.tile([128, L * CW8], f8)
        wt16 = wpool.tile([128, L * CW16], f16)
        ct = wpool.tile([128, CONST_COLS], f16)
        bt = wpool.tile([128, L * CB], f32)

        # PE warmup: keep the clock-ramp monitor busy while DMAs land
        warm = wpool.tile([128, 128], f16)
        wps = hpool.tile([128, WARM_N], f32, tag="hp")
        nc.gpsimd.memset(warm[:], 0.0)
        for _ in range(WARM_MMS):
            nc.tensor.matmul(wps[:, 0:WARM_N], warm[:, 0:WARM_N],
                             warm[:, 0:WARM_N], start=True, stop=True)

        nc.sync.dma_start(wt8[:, 0:CW8], w8_d[0])
        nc.sync.dma_start(ct[:], cn_d[:])
        nc.sync.dma_start(wt16[:], w16_d[:])
        nc.sync.dma_start(bt[:], b_d[:])

        y16s, y8s = [], []
        for p in range(NP):
            yt = wpool.tile([128, BT], f16, name=f"y16_{p}")
            nc.sync.dma_start(yt[0:64, :], u16_d[:, (2 * p) * BT:(2 * p + 1) * BT])
            nc.sync.dma_start(yt[64:128, :],
                              u16_d[:, (2 * p + 1) * BT:(2 * p + 2) * BT])
            y16s.append(yt)
        for p in range(NP):
            yt = wpool.tile([128, 2, BT], f8, name=f"y8_{p}")
            nc.sync.dma_start(yt[0:64, 0, :], u8_d[:, (2 * p) * BT:(2 * p + 1) * BT])
            nc.sync.dma_start(yt[64:128, 0, :],
                              u8_d[:, (2 * p + 1) * BT:(2 * p + 2) * BT])
            nc.gpsimd.memset(yt[:, 1, :], 1.0)
            y8s.append(yt)
        for l in range(1, L):
            nc.sync.dma_start(wt8[:, l * CW8:(l + 1) * CW8], w8_d[l])

        pairst = {}
        pools = (hpool, lppool, h8pool, epool, opool, pairst)
        tiles = (wt8, wt16, ct, bt, y16s, y8s)
        for l in range(L):
            for t in range(NT):
                o32 = _emit_layer(nc, tc, pools, tiles, l, t, l == L - 1)
                if o32 is not None:
                    p = t >> 1
                    nc.sync.dma_start(out_d[:, (2 * p) * BT:(2 * p + 1) * BT],
                                      o32[0:64, :])
                    nc.sync.dma_start(out_d[:, (2 * p + 1) * BT:(2 * p + 2) * BT],
                                      o32[64:128, :])
    _split_excess_waits(nc, maxw=1)
    return nc


_NC_CACHE = None


def _prep_blobs(inputs):
    M0, M1, M2 = _build_masks()
    w8 = np.zeros((L, 128, CW8), F8)
    w16 = np.zeros((128, L * CW16), np.float16)
    cn = np.zeros((128, CONST_COLS), np.float16)
    bb = np.zeros((128, L * CB), np.float32)
    cn[0:64, 0:64] = np.eye(64, dtype=np.float16)
    cn[64:128, 0:64] = np.eye(64, dtype=np.float16)
    cn[:, 64:CONST_COLS] = 1.0
    for l in range(L):
        for n, name in ((0, "loc"), (1, "scale")):
            A0 = (M0[l] * inputs[f"{name}_W0"][l]).astype(np.float32).T  # [64,256]
            A1 = (M1[l] * inputs[f"{name}_W1"][l]).astype(np.float32).T  # [256,256]
            A2 = (M2[l] * inputs[f"{name}_W2"][l]).astype(np.float32).T  # [256,64]
            b0 = inputs[f"{name}_b0"][l].astype(np.float32)
            b1 = inputs[f"{name}_b1"][l].astype(np.float32)
            b2 = inputs[f"{name}_b2"][l].astype(np.float32)
            if n == 0:
                A2 = -A2
                w16[0, l * CW16: l * CW16 + 64] = -b2.astype(np.float16)
                w16[64, l * CW16: l * CW16 + 64] = -b2.astype(np.float16)
            else:
                bb[0:64, l * CB + 4] = -b2
                bb[64:128, l * CB + 4] = -b2
            for m in (0, 1):
                off = n * 512 + m * 256
                a0c = A0[:, m * 128:(m + 1) * 128].astype(F8)
                w8[l, 0:64, off: off + 128] = a0c
                w8[l, 64:128, off: off + 128] = a0c
                b0c = b0[m * 128:(m + 1) * 128].astype(F8)
                w8[l, 0, off + 128: off + 256] = b0c
                w8[l, 64, off + 128: off + 256] = b0c
                off = 1024 + n * 512 + m * 256
                w8[l, :, off: off + 128] = \
                    A1[0:128, m * 128:(m + 1) * 128].astype(F8)
                w8[l, :, off + 128: off + 256] = \
                    A1[128:256, m * 128:(m + 1) * 128].astype(F8)
                bb[:, l * CB + n * 2 + m] = b1[m * 128:(m + 1) * 128]
            off = 2048 + n * 128
            w8[l, :, off: off + 64] = A2[0:128, :].astype(F8)
            w8[l, :, off + 64: off + 128] = A2[128:256, :].astype(F8)
    return w8, w16, cn, bb


def make_in_maps(inputs):
    inputs = {k: np.asarray(v) for k, v in inputs.items()}
    u = inputs["u"].astype(np.float32)            # [B, 64]
    w8, w16, cn, bb = _prep_blobs(inputs)
    uT16 = np.ascontiguousarray(u.T).astype(np.float16)
    uT8 = uT16.astype(F8)
    in_maps = []
    for c in range(NCORES):
        sl = slice(c * BC, (c + 1) * BC)
        in_maps.append({
            "u16": np.ascontiguousarray(uT16[:, sl]),
            "u8": np.ascontiguousarray(uT8[:, sl]),
            "w8": w8, "w16": w16, "cn": cn, "bias": bb,
        })
    return in_maps


def kernel(**inputs):
    global _NC_CACHE
    if _NC_CACHE is None:
        _NC_CACHE = _build()
    nc = _NC_CACHE
    in_maps = make_in_maps(inputs)
    res = run_bass_kernel_spmd(nc, in_maps, core_ids=list(range(NCORES)))
    out = np.empty((64, B), np.float32)
    for c in range(NCORES):
        out[:, c * BC:(c + 1) * BC] = res.results[c]["out"]
    return np.ascontiguousarray(out.T)
